# revision 12
# baseline (speedup 1.0000x reference)
"""GRCNN 3-step message-passing kernel for 8 Trainium2 NeuronCores.

Strategy: pure data parallel over batch (64 -> 8 elems/core). Per batch
element every operation is a 512x512x512 matmul (30 of them), executed on
the PE array in fp16 with fp32 PSUM accumulation. Host pre-transposes the
adjacency-type matrices so all device matmuls are transpose-free, and
folds the D^-1 row normalization and the row masks into per-partition
scalars applied during PSUM evacuation with fused scalar_tensor_tensor.
"""

import sys

if '/opt/trn_rl_repo' not in sys.path:
    sys.path.insert(0, '/opt/trn_rl_repo')

import numpy as np

B, N, R, D = 64, 512, 512, 512
N_CORES = 8
BPC = B // N_CORES  # batch elems per core
P = 128             # partitions
T = 4               # 512 / 128 partition tiles
EPS = 1e-10
ALPHA = 2.0 ** -4   # input scale; relu network is positively homogeneous

# Per-step message configs.
#  node side: list of (moving source, weight idx, scal col base), n2r handled
#  separately (stationary = rela).  scal col bases: A1=0, A2=4, A3=8, am=12, rm=16.
_STEP_AMSGS = [["b1", "b2", "b3"], ["b1", "b2"], ["b1"]]
_AMSG_W = [[0, 1, 2], [3, 4], [5]]          # Wnn index per step per A-msg
_AMSG_SCOL = {"b1": 0, "b2": 4, "b3": 8}
_N2R_W = [6, 7, 8]                           # Wnr indices (offset in wts)
_RS_W = [9, 11, 13]                          # Wr[s,0]
_RO_W = [10, 12, 14]                         # Wr[s,1]

# Optional: test harness sets this to a context-manager factory to wrap the
# device execution (e.g. NTFF profiling). Not used by the grader.
PROFILE = None
_NC_CACHE = {}


def _build_program(with_bias):
    from concourse import bacc
    import concourse.mybir as mybir
    import concourse.tile as tile

    F16 = mybir.dt.float16
    F32 = mybir.dt.float32
    MULT = mybir.AluOpType.mult
    ADD = mybir.AluOpType.add
    RELU = mybir.ActivationFunctionType.Relu

    nc = bacc.Bacc("TRN2", target_bir_lowering=False, debug=False,
                   num_devices=N_CORES)

    def inp(name, shape, dt=F16):
        return nc.declare_dram_parameter(name, list(shape), dt, isOutput=False)

    elem_shape = (BPC, P, T, 512)
    p_b1 = inp("b1", elem_shape)
    p_b2 = inp("b2", elem_shape)
    p_b3 = inp("b3", elem_shape)
    p_s = inp("s", elem_shape)
    p_o = inp("o", elem_shape)
    p_q = inp("q", elem_shape)
    p_x = inp("x", elem_shape)
    p_r = inp("r", elem_shape)
    p_w = inp("w", (P, 15 * T, 512))
    p_sc = inp("sc", (BPC, P, 20), F32)
    if with_bias:
        p_bn = inp("bias_n", (P, 3, 512), F32)
        p_br = inp("bias_r", (P, 3, 512), F32)
    p_on = nc.declare_dram_parameter("onode", list(elem_shape), F32, isOutput=True)
    p_or = nc.declare_dram_parameter("orela", list(elem_shape), F32, isOutput=True)

    movings = {"b1": p_b1, "b2": p_b2, "b3": p_b3, "s": p_s, "o": p_o}

    with tile.TileContext(nc) as tc:
        with (
            tc.tile_pool(name="wpool", bufs=1) as wpool,
            tc.tile_pool(name="inpool", bufs=2) as inpool,
            tc.tile_pool(name="mpool", bufs=1) as mpool,
            tc.tile_pool(name="xpool", bufs=2) as xpool,
            tc.tile_pool(name="apool", bufs=2) as apool,
            tc.tile_pool(name="stpool", bufs=3) as stpool,
            tc.tile_pool(name="pspool", bufs=8, space="PSUM") as pspool,
        ):
            wt = wpool.tile([P, 15 * T, 512], F16)
            nc.sync.dma_start(out=wt[:], in_=p_w[:])
            if with_bias:
                bn_t = wpool.tile([P, 3, 512], F32, tag="bias_n")
                br_t = wpool.tile([P, 3, 512], F32, tag="bias_r")
                nc.sync.dma_start(out=bn_t[:], in_=p_bn[:])
                nc.sync.dma_start(out=br_t[:], in_=p_br[:])

            for b in range(BPC):
                mv = {}
                for name, par in movings.items():
                    t_ = inpool.tile([P, T, 512], F16, tag=name)
                    nc.sync.dma_start(out=t_[:], in_=par[b])
                    mv[name] = t_
                qt = inpool.tile([P, T, 512], F16, tag="q")
                nc.sync.dma_start(out=qt[:], in_=p_q[b])
                x = inpool.tile([P, T, 512], F16, tag="x")
                nc.sync.dma_start(out=x[:], in_=p_x[b])
                r = inpool.tile([P, T, 512], F16, tag="r")
                nc.sync.dma_start(out=r[:], in_=p_r[b])
                sc = inpool.tile([P, 20], F32, tag="sc")
                nc.sync.dma_start(out=sc[:], in_=p_sc[b])

                for s in range(3):
                    amsgs = _STEP_AMSGS[s]
                    last = s == 2

                    # ---- messages with stationary = x: A-msgs + S + O ----
                    names = amsgs + ["s", "o"]
                    mt = {n: mpool.tile([P, T, 512], F16, tag=f"m_{n}", name=f"m_{n}_{b}_{s}")
                          for n in names}
                    ei = 0
                    for td in range(T):
                        for n in names:
                            ps = pspool.tile([P, 512], F32, tag="ps", name=f"ps_{b}_{s}_{td}_{n}")
                            for tj in range(T):
                                nc.tensor.matmul(
                                    ps[:], x[:, tj, td * P:(td + 1) * P],
                                    mv[n][:, tj, :],
                                    start=(tj == 0), stop=(tj == T - 1))
                            nc.scalar.copy(out=mt[n][:, td, :], in_=ps[:])
                            ei += 1

                    # ---- n2r message: stationary = r, moving = Q ----
                    mq = mpool.tile([P, T, 512], F16, tag="m_q")
                    for td in range(T):
                        ps = pspool.tile([P, 512], F32, tag="ps")
                        for tj in range(T):
                            nc.tensor.matmul(
                                ps[:], r[:, tj, td * P:(td + 1) * P],
                                qt[:, tj, :],
                                start=(tj == 0), stop=(tj == T - 1))
                        nc.scalar.copy(out=mq[:, td, :], in_=ps[:])

                    # ---- node linears; accumulate into fp32 acc ----
                    # order: n2r first (init with x residual), then A-msgs
                    lin = [(mq, _N2R_W[s], 12)]
                    lin += [(mt[n], _AMSG_W[s][i], _AMSG_SCOL[n])
                            for i, n in enumerate(amsgs)]
                    acc = apool.tile([P, T, 512], F32, tag="acc")
                    if not last:
                        x_next = xpool.tile([P, T, 512], F16, tag="xn")
                    else:
                        x_next = stpool.tile([P, T, 512], F32, tag="stage", name=f"xstage_{b}")
                    for tn in range(T):
                        for i, (mtile, wix, col) in enumerate(lin):
                            pl = pspool.tile([P, 512], F32, tag="ps")
                            for td in range(T):
                                nc.tensor.matmul(
                                    pl[:], mtile[:, td, tn * P:(tn + 1) * P],
                                    wt[:, wix * T + td, :],
                                    start=(td == 0), stop=(td == T - 1))
                            in1 = x[:, tn, :] if i == 0 else acc[:, tn, :]
                            nc.vector.scalar_tensor_tensor(
                                out=acc[:, tn, :], in0=pl[:],
                                scalar=sc[:, col + tn:col + tn + 1],
                                in1=in1, op0=MULT, op1=ADD)
                        if with_bias:
                            nc.vector.scalar_tensor_tensor(
                                out=acc[:, tn, :], in0=bn_t[:, s, :],
                                scalar=sc[:, 12 + tn:12 + tn + 1],
                                in1=acc[:, tn, :], op0=MULT, op1=ADD)
                        nc.scalar.activation(x_next[:, tn, :], acc[:, tn, :],
                                             RELU)
                    if last:
                        nc.gpsimd.dma_start(out=p_on[b], in_=x_next[:])

                    # ---- rela linears: rs + ro accumulate in one psum ----
                    if not last:
                        r_next = xpool.tile([P, T, 512], F16, tag="rn")
                    else:
                        r_next = stpool.tile([P, T, 512], F32, tag="stage", name=f"rstage_{b}")
                    for tr in range(T):
                        pr = pspool.tile([P, 512], F32, tag="ps")
                        for td in range(T):
                            nc.tensor.matmul(
                                pr[:], mt["s"][:, td, tr * P:(tr + 1) * P],
                                wt[:, _RS_W[s] * T + td, :],
                                start=(td == 0), stop=False)
                        for td in range(T):
                            nc.tensor.matmul(
                                pr[:], mt["o"][:, td, tr * P:(tr + 1) * P],
                                wt[:, _RO_W[s] * T + td, :],
                                start=False, stop=(td == T - 1))
                        if with_bias:
                            tmp = stpool.tile([P, T, 512], F32, tag="stage", name=f"rbias_{b}_{s}_{tr}")
                            nc.vector.scalar_tensor_tensor(
                                out=tmp[:, tr, :], in0=br_t[:, s, :],
                                scalar=sc[:, 16 + tr:16 + tr + 1],
                                in1=r[:, tr, :], op0=MULT, op1=ADD)
                            in1 = tmp[:, tr, :]
                        else:
                            in1 = r[:, tr, :]
                        nc.vector.scalar_tensor_tensor(
                            out=r_next[:, tr, :], in0=pr[:],
                            scalar=sc[:, 16 + tr:16 + tr + 1],
                            in1=in1, op0=MULT, op1=ADD)
                        nc.vector.tensor_relu(r_next[:, tr, :], r_next[:, tr, :])
                    if last:
                        nc.gpsimd.dma_start(out=p_or[b], in_=r_next[:])

                    x, r = x_next, r_next

    nc.compile()
    return nc


def _pack_tiles(a16):
    """[B, 512, free] -> [B, 128, 4, free] with row = t*128 + p."""
    b, n, f = a16.shape
    return np.ascontiguousarray(
        a16.reshape(b, T, P, f).transpose(0, 2, 1, 3))


def _cols(v):
    """[B, 512] fp32 -> [B, 128, 4] per-partition scalar columns."""
    return v.reshape(B, T, P).transpose(0, 2, 1)


def kernel(node, rela, p_att_masks, p_rela_masks, adj1, adj2, adj3,
           rela_sub, rela_obj, rela_n2r, Wnn, bnn, Wnr, bnr, Wr, br):
    from concourse.bass_utils import run_bass_kernel_spmd

    f16 = np.float16

    def tp(a):  # batch transpose -> fp16 -> tile pack
        return _pack_tiles(np.ascontiguousarray(
            np.asarray(a, np.float32).transpose(0, 2, 1)).astype(f16))

    b1 = tp(adj1)
    b2 = tp(adj2)
    b3 = tp(adj3)
    s_ = tp(rela_sub)
    o_ = tp(rela_obj)
    q_ = tp(rela_n2r)
    x_ = _pack_tiles((np.asarray(node, np.float32) * ALPHA).astype(f16))
    r_ = _pack_tiles((np.asarray(rela, np.float32) * ALPHA).astype(f16))

    # weights: transpose each [o,d] -> [d,o], pack [128, 15*4, 512] fp16
    w_all = np.concatenate([
        np.asarray(Wnn, np.float32),
        np.asarray(Wnr, np.float32),
        np.asarray(Wr, np.float32).reshape(6, D, D),
    ], axis=0).transpose(0, 2, 1)                       # [15, d, o]
    w_dev = np.ascontiguousarray(
        w_all.reshape(15, T, P, D).transpose(2, 0, 1, 3)
    ).reshape(P, 15 * T, D).astype(f16)

    am = np.asarray(p_att_masks, np.float32)
    rm = np.asarray(p_rela_masks, np.float32)
    scal = np.empty((B, P, 20), np.float32)
    for m, adj in enumerate([adj1, adj2, adj3]):
        dinv = 1.0 / (np.asarray(adj, np.float32).sum(axis=2) + EPS)
        scal[:, :, m * 4:(m + 1) * 4] = _cols(dinv * am)
    scal[:, :, 12:16] = _cols(am)
    scal[:, :, 16:20] = _cols(rm)

    with_bias = bool(np.any(np.asarray(bnn)) or np.any(np.asarray(bnr))
                     or np.any(np.asarray(br)))
    if with_bias:
        # per-step node bias sums and rela bias sums, replicated on partitions
        bnn_f = np.asarray(bnn, np.float32)
        bnr_f = np.asarray(bnr, np.float32)
        br_f = np.asarray(br, np.float32)
        n_sums = [bnn_f[0] + bnn_f[1] + bnn_f[2] + bnr_f[0],
                  bnn_f[3] + bnn_f[4] + bnr_f[1],
                  bnn_f[5] + bnr_f[2]]
        r_sums = [br_f[0, 0] + br_f[0, 1], br_f[1, 0] + br_f[1, 1],
                  br_f[2, 0] + br_f[2, 1]]
        bn_host = np.ascontiguousarray(
            np.broadcast_to(np.stack(n_sums)[None, :, :], (P, 3, D))) * ALPHA
        br_host = np.ascontiguousarray(
            np.broadcast_to(np.stack(r_sums)[None, :, :], (P, 3, D))) * ALPHA

    nc = _NC_CACHE.get(with_bias)
    if nc is None:
        nc = _NC_CACHE[with_bias] = _build_program(with_bias)

    in_maps = []
    for c in range(N_CORES):
        sl = slice(c * BPC, (c + 1) * BPC)
        m = {"b1": b1[sl], "b2": b2[sl], "b3": b3[sl], "s": s_[sl],
             "o": o_[sl], "q": q_[sl], "x": x_[sl], "r": r_[sl],
             "w": w_dev, "sc": scal[sl]}
        if with_bias:
            m["bias_n"] = bn_host
            m["bias_r"] = br_host
        in_maps.append(m)

    if PROFILE is not None:
        with PROFILE():
            res = run_bass_kernel_spmd(nc, in_maps, list(range(N_CORES)))
    else:
        res = run_bass_kernel_spmd(nc, in_maps, list(range(N_CORES)))

    def unpack(key):
        full = np.concatenate([res.results[c][key] for c in range(N_CORES)], 0)
        # [B, 128, 4, 512] -> [B, 512, 512]
        out = np.ascontiguousarray(
            full.transpose(0, 2, 1, 3)).reshape(B, N, D)
        out *= 1.0 / ALPHA
        return out

    return (unpack("onode"), unpack("orela"))
